# revision 1
# baseline (speedup 1.0000x reference)
"""HGCN (hypergraph conv net) Trainium2 kernel over 8 NeuronCores.

Math: out = log_softmax(pool_mean(h2) @ Wfc + bfc) where
  h_k = relu((D^-1 H B^-1 H^T h_{k-1}) @ Wk + bk)   (W commutes with the linear aggregation)

Strategy (per core, nodes sharded 12500/core):
  - CPU pre-pass bins incidence entries (sorted by destination) into 128-entry
    tiles with <=32 distinct destinations; a one-hot (weight-valued) selection
    matrix S is built on DVE via iota-compare and the segment-sum becomes
    psum += S^T G on PE.  4 tiles share a psum bank at 32-column offsets, so
    every destination lands in exactly one psum column -> scatter indices are
    globally unique (dma_scatter_add races on duplicate rows!).
  - stage A (node->edge): gather rows of the node tensor, combine with
    Binv-weighted S, scatter unique edge rows into a dense partial e.
  - AllReduce e across the 8 cores (bf16).
  - stage B (edge->node): gather e rows, combine with Dinv-weighted S,
    scatter unique local-node rows into dense A.
  - W-phase: per 128-node tile: PE-transpose A, matmul with W, +bias, relu.
    Layer-2 W-phase also accumulates pooling psum += h2^T @ S_pool.
  - AllReduce pooled sums, tiny fc + log_softmax, output [64,16] f32.

Everything index-derived is precomputed on CPU (int16 token streams for the
SWDGE gather/scatter, f32 col-id/weight streams for the S builds).
"""
import os
import numpy as np

import concourse.bacc as bacc
import concourse.tile as tile
import concourse.mybir as mybir
from concourse.bass_utils import run_bass_kernel_spmd

NCORES = 8
N_NODES = 100000
N_EDGES = 20000
NNZ = 600000
N_GRAPHS = 64
C = 128
OUT_C = 16
NPC = N_NODES // NCORES          # 12500 nodes per core
NPC_PAD = 12544                  # 98 * 128
NTILES_N = NPC_PAD // 128        # 98
E_PAD = 20096                    # 157 * 128 (rows >= 20000 are trash)
E_TRASH = 20000
N_TRASH = 12500                  # rows 12500..12543 of dense node tensors

K_DST = 32                       # max distinct destinations per 128-entry tile
GCALL_TILES = 8                  # tiles per dma_gather call (1024 tokens; ring cap)
SCH_GROUPS = 4                   # psum groups per dma_scatter_add call (512 tokens)
NQ = 4                           # SWDGE queues (ucode max)

F32 = mybir.dt.float32
BF16 = mybir.dt.bfloat16
I16 = mybir.dt.int16


# ----------------------------------------------------------------- CPU pack
def _pack_stage(dst, gat, wgt, n_dst_rows, trash_row):
    """Bin-pack entries (any order) for the tiled segment-sum.

    Returns (gidx[NT*128] int32 gather rows, colid[NT,128] f32, w[NT,128] f32,
             scat[NG*128] int32 scatter rows, NT) with NT a multiple of 4;
    caller pads NT across cores.
    """
    order = np.argsort(dst, kind="stable")
    dst = dst[order]; gat = gat[order]; wgt = wgt[order]
    n = len(dst)
    # run-length encode destinations
    boundaries = np.flatnonzero(np.diff(dst)) + 1
    starts = np.concatenate(([0], boundaries))
    counts = np.diff(np.concatenate((starts, [n])))
    # greedy binning: <=128 entries and <=K_DST dsts per bin
    bins = []  # list of lists of run indices
    cur, cur_e, cur_d = [], 0, 0
    for r in range(len(starts)):
        c = int(counts[r])
        if cur and (cur_e + c > 128 or cur_d + 1 > K_DST):
            bins.append(cur)
            cur, cur_e, cur_d = [], 0, 0
        # a run longer than 128 entries cannot happen (max degree << 128)
        cur.append(r); cur_e += c; cur_d += 1
    if cur:
        bins.append(cur)
    nt = len(bins)
    gidx = np.zeros((nt, 128), np.int32)
    colid = np.full((nt, 128), -1.0, np.float32)
    w = np.zeros((nt, 128), np.float32)
    # scatter rows per (group, col); NG groups of 128 cols
    ng = (nt + 3) // 4
    scat = np.full((ng, 128), trash_row, np.int32)
    for b, runs in enumerate(bins):
        off = 32 * (b % 4)
        g = b // 4
        pos = 0
        for rank, r in enumerate(runs):
            s, c = int(starts[r]), int(counts[r])
            gidx[b, pos:pos + c] = gat[s:s + c]
            colid[b, pos:pos + c] = off + rank
            w[b, pos:pos + c] = wgt[s:s + c]
            scat[g, off + rank] = dst[s]
            pos += c
    return gidx, colid, w, scat, nt


def _wrap16(tokens):
    """[T] int -> [128, T//16] int16, 16-partition wrap replicated 8x."""
    a = tokens.reshape(-1, 16).T.astype(np.int16)
    return np.tile(a, (8, 1)).copy()


def _prep_inputs(x, node_idx, edge_idx, batch, W1, b1, W2, b2, Wfc, bfc):
    node_idx = np.asarray(node_idx).astype(np.int64)
    edge_idx = np.asarray(edge_idx).astype(np.int64)
    batch = np.asarray(batch).astype(np.int64)
    x = np.asarray(x, np.float32)

    De = np.bincount(edge_idx, minlength=N_EDGES).astype(np.float32)
    Dn = np.bincount(node_idx, minlength=N_NODES).astype(np.float32)
    Binv = np.where(De > 0, 1.0 / np.maximum(De, 1), 0.0).astype(np.float32)
    Dinv = np.where(Dn > 0, 1.0 / np.maximum(Dn, 1), 0.0).astype(np.float32)
    cnt = np.bincount(batch, minlength=N_GRAPHS).astype(np.float32)
    cntinv = (1.0 / np.maximum(cnt, 1.0)).astype(np.float32)

    owner = node_idx // NPC
    packs = []
    for c in range(NCORES):
        m = owner == c
        ln = (node_idx[m] - c * NPC).astype(np.int64)
        le = edge_idx[m]
        # stage A: dst=edge, gather=local node rows, weight=Binv[edge]
        pa = _pack_stage(le, ln, Binv[le], N_EDGES, E_TRASH)
        # stage B: dst=local node, gather=edge rows, weight=Dinv[node global]
        pb = _pack_stage(ln, le, Dinv[node_idx[m]], NPC, N_TRASH)
        packs.append((pa, pb))

    def _round_nt(v):
        return -(-v // GCALL_TILES) * GCALL_TILES

    NT_A = _round_nt(max(p[0][4] for p in packs))
    NT_B = _round_nt(max(p[1][4] for p in packs))
    # scatter chunks of SCH_GROUPS groups = SCH_GROUPS*4 tiles; NT multiple of 128
    NT_A = -(-NT_A // (SCH_GROUPS * 4)) * (SCH_GROUPS * 4)
    NT_B = -(-NT_B // (SCH_GROUPS * 4)) * (SCH_GROUPS * 4)

    def _pad_pack(p, NT, trash_row):
        gidx, colid, w, scat, nt = p
        ng, NG = (nt + 3) // 4, NT // 4
        g2 = np.zeros((NT, 128), np.int32); g2[:nt] = gidx
        c2 = np.full((NT, 128), -1.0, np.float32); c2[:nt] = colid
        w2 = np.zeros((NT, 128), np.float32); w2[:nt] = w
        s2 = np.full((NG, 128), trash_row, np.int32); s2[:ng] = scat
        return g2, c2, w2, s2

    bx = x.astype(np.float32)
    in_maps = []
    meta = dict(NT_A=NT_A, NT_B=NT_B)
    for c in range(NCORES):
        pa, pb = packs[c]
        gA, cA, wA, sA = _pad_pack(pa, NT_A, E_TRASH)
        gB, cB, wB, sB = _pad_pack(pb, NT_B, N_TRASH)
        xc = np.zeros((NPC_PAD, C), np.float32)
        xc[:NPC] = bx[c * NPC:(c + 1) * NPC]
        batchcol = np.full((NTILES_N, 128), -1.0, np.float32)
        bc = batch[c * NPC:(c + 1) * NPC].astype(np.float32)
        batchcol.reshape(-1)[:NPC] = bc
        im = {
            "x": xc.astype(mybir.dt.np(BF16)),
            "gA": _wrap16(gA.reshape(-1)), "sA": _wrap16(sA.reshape(-1)),
            "gB": _wrap16(gB.reshape(-1)), "sB": _wrap16(sB.reshape(-1)),
            "colA": cA.T.copy(), "wA": wA.T.copy(),
            "colB": cB.T.copy(), "wB": wB.T.copy(),
            "batchcol": batchcol.T.copy(),
            "W1": np.asarray(W1, np.float32).astype(mybir.dt.np(BF16)),
            "W2": np.asarray(W2, np.float32).astype(mybir.dt.np(BF16)),
            "Wfc": np.asarray(Wfc, np.float32),
            "b1rep": np.tile(np.asarray(b1, np.float32)[None, :], (128, 1)),
            "b2rep": np.tile(np.asarray(b2, np.float32)[None, :], (128, 1)),
            "bfcrep": np.tile(np.asarray(bfc, np.float32)[None, :], (N_GRAPHS, 1)),
            "cntinvrep": np.tile(cntinv[None, :], (128, 1)),
            "eye": np.eye(128, dtype=mybir.dt.np(BF16)),
        }
        in_maps.append(im)
    return in_maps, meta


# ----------------------------------------------------------------- device
def _build(meta):
    NT_A, NT_B = meta["NT_A"], meta["NT_B"]
    NG_A, NG_B = NT_A // 4, NT_B // 4

    nc = bacc.Bacc("TRN2", target_bir_lowering=False, debug=False,
                   num_devices=NCORES, num_swdge_queues=NQ)

    def din(name, shape, dt):
        return nc.dram_tensor(name, shape, dt, kind="ExternalInput")

    x_t = din("x", [NPC_PAD, C], BF16)
    gA_t = din("gA", [128, NT_A * 8], I16)
    sA_t = din("sA", [128, NG_A * 8], I16)
    gB_t = din("gB", [128, NT_B * 8], I16)
    sB_t = din("sB", [128, NG_B * 8], I16)
    colA_t = din("colA", [128, NT_A], F32)
    wA_t = din("wA", [128, NT_A], F32)
    colB_t = din("colB", [128, NT_B], F32)
    wB_t = din("wB", [128, NT_B], F32)
    batchcol_t = din("batchcol", [128, NTILES_N], F32)
    W1_t = din("W1", [C, C], BF16)
    W2_t = din("W2", [C, C], BF16)
    Wfc_t = din("Wfc", [C, OUT_C], F32)
    b1rep_t = din("b1rep", [128, C], F32)
    b2rep_t = din("b2rep", [128, C], F32)
    bfcrep_t = din("bfcrep", [N_GRAPHS, OUT_C], F32)
    cntinvrep_t = din("cntinvrep", [128, N_GRAPHS], F32)
    eye_t = din("eye", [128, 128], BF16)

    out_t = nc.dram_tensor("out", [N_GRAPHS, OUT_C], F32, kind="ExternalOutput")

    # internal dram
    e_part = [nc.dram_tensor(f"e_part{i}", [E_PAD, C], BF16) for i in range(2)]
    e_full = [nc.dram_tensor(f"e_full{i}", [E_PAD, C], BF16, addr_space="Shared")
              for i in range(2)]
    a_dense = [nc.dram_tensor(f"a_dense{i}", [NPC_PAD, C], BF16) for i in range(2)]
    h1 = nc.dram_tensor("h1", [NPC_PAD, C], BF16)
    pool_part = nc.dram_tensor("pool_part", [128, N_GRAPHS], F32)
    pool_full = nc.dram_tensor("pool_full", [128, N_GRAPHS], F32,
                               addr_space="Shared")

    with tile.TileContext(nc) as tc:
        with (
            tc.tile_pool(name="res", bufs=1) as res,
            tc.tile_pool(name="gp", bufs=2) as gp,
            tc.tile_pool(name="sp", bufs=2) as sp,
            tc.tile_pool(name="stp", bufs=2) as stp,
            tc.tile_pool(name="wp", bufs=3) as wp,
            tc.tile_pool(name="ps", bufs=2, space="PSUM") as ps,
            tc.tile_pool(name="psw", bufs=2, space="PSUM") as psw,
            tc.tile_pool(name="pspool", bufs=1, space="PSUM") as pspool,
        ):
            # ---- resident loads
            def rload(t, shape, dt, eng=nc.sync):
                tl = res.tile(shape, dt, tag=t.name)
                eng.dma_start(tl[:], t.ap())
                return tl

            gA = rload(gA_t, [128, NT_A * 8], I16)
            sA = rload(sA_t, [128, NG_A * 8], I16)
            gB = rload(gB_t, [128, NT_B * 8], I16)
            sB = rload(sB_t, [128, NG_B * 8], I16)
            colA = rload(colA_t, [128, NT_A], F32)
            wA = rload(wA_t, [128, NT_A], F32)
            colB = rload(colB_t, [128, NT_B], F32)
            wB = rload(wB_t, [128, NT_B], F32)
            batchcol = rload(batchcol_t, [128, NTILES_N], F32)
            W1 = rload(W1_t, [C, C], BF16)
            W2 = rload(W2_t, [C, C], BF16)
            Wfc = rload(Wfc_t, [C, OUT_C], F32)
            b1rep = rload(b1rep_t, [128, C], F32)
            b2rep = rload(b2rep_t, [128, C], F32)
            bfcrep = rload(bfcrep_t, [N_GRAPHS, OUT_C], F32)
            cntinvrep = rload(cntinvrep_t, [128, N_GRAPHS], F32)
            eye = rload(eye_t, [128, 128], BF16)

            iota = res.tile([128, 128], BF16, tag="iota")
            nc.gpsimd.iota(iota[:], [[1, 128]], channel_multiplier=0,
                           allow_small_or_imprecise_dtypes=True)
            zt = res.tile([128, 2048], BF16, tag="zt")
            nc.gpsimd.memset(zt[:], 0.0)

            def dram_memset(t, nrows):
                total = nrows * C
                done = 0
                while done < total:
                    n = min(262144, total - done)
                    r0, r1 = done // C, (done + n) // C
                    nc.sync.dma_start(t.ap()[r0:r1, :], zt[:, 0:n // 128])
                    done += n

            def stage(src_ap, gidx, col, wv, sidx, dst, NT):
                """Tiled segment-sum: dst[scat] += S^T gather(src)."""
                n_gcall = NT // GCALL_TILES
                gcall_per_sc = (SCH_GROUPS * 4) // GCALL_TILES  # 2
                gcols = GCALL_TILES * 8       # idx cols per gather call
                st = None
                for k in range(n_gcall):
                    if k % gcall_per_sc == 0:
                        st = stp.tile([128, SCH_GROUPS, 128], BF16, tag="st")
                    gt = gp.tile([128, GCALL_TILES, 128], BF16, tag="g")
                    nc.gpsimd.dma_gather(
                        gt[:], src_ap, gidx[:, k * gcols:(k + 1) * gcols],
                        GCALL_TILES * 128, GCALL_TILES * 128, C,
                        queue_num=k % NQ)
                    for u in range(GCALL_TILES // 4):     # psum groups
                        acc = ps.tile([128, 128], F32, tag="agg")
                        for j in range(4):
                            t = k * GCALL_TILES + 4 * u + j
                            S = sp.tile([128, 128], BF16, tag="S")
                            nc.vector.tensor_scalar(
                                S[:], iota[:], col[:, t:t + 1], wv[:, t:t + 1],
                                op0=mybir.AluOpType.is_equal,
                                op1=mybir.AluOpType.mult)
                            nc.tensor.matmul(acc[:], S[:], gt[:, 4 * u + j, :],
                                             start=(j == 0), stop=(j == 3))
                        gslot = (k % gcall_per_sc) * (GCALL_TILES // 4) + u
                        nc.vector.tensor_copy(st[:, gslot, :], acc[:])
                    if k % gcall_per_sc == gcall_per_sc - 1:
                        m = k // gcall_per_sc
                        nc.gpsimd.dma_scatter_add(
                            dst.ap(), st[:],
                            sidx[:, m * (SCH_GROUPS * 8):(m + 1) * (SCH_GROUPS * 8)],
                            SCH_GROUPS * 128, SCH_GROUPS * 128, C,
                            queue_num=m % NQ)

            def w_phase(a_t, W, brep, h_out, pool_acc):
                for j in range(NTILES_N):
                    at = wp.tile([128, 128], BF16, tag="a")
                    nc.sync.dma_start(at[:], a_t.ap()[j * 128:(j + 1) * 128, :])
                    tp = psw.tile([128, 128], BF16, tag="tp")
                    nc.tensor.transpose(tp[:], at[:], eye[:])
                    ats = wp.tile([128, 128], BF16, tag="ats")
                    nc.vector.tensor_copy(ats[:], tp[:])
                    hp = psw.tile([128, 128], F32, tag="hp")
                    nc.tensor.matmul(hp[:], ats[:], W[:], start=True, stop=True)
                    ht = wp.tile([128, 128], BF16, tag="ht")
                    nc.vector.tensor_tensor(ht[:], hp[:], brep[:],
                                            op=mybir.AluOpType.add)
                    nc.vector.tensor_scalar_max(ht[:], ht[:], 0.0)
                    if h_out is not None:
                        nc.sync.dma_start(h_out.ap()[j * 128:(j + 1) * 128, :],
                                          ht[:])
                    if pool_acc is not None:
                        Sp = sp.tile([128, N_GRAPHS], BF16, tag="Spool")
                        nc.vector.tensor_scalar(
                            Sp[:], iota[:, 0:N_GRAPHS], batchcol[:, j:j + 1],
                            None, op0=mybir.AluOpType.is_equal)
                        nc.tensor.matmul(pool_acc, ht[:], Sp[:],
                                         start=(j == 0), stop=(j == NTILES_N - 1),
                                         skip_group_check=True)

            SA = int(os.environ.get("STOP_AFTER", "99"))
            # ---------------- layer 1
            dram_memset(e_part[0], E_PAD)
            dram_memset(a_dense[0], NPC_PAD)
            if SA >= 1:
                stage(x_t.ap(), gA, colA, wA, sA, e_part[0], NT_A)
            if SA >= 2:
                nc.gpsimd.collective_compute(
                    "AllReduce", mybir.AluOpType.add,
                    replica_groups=[list(range(NCORES))],
                    ins=[e_part[0].ap()], outs=[e_full[0].ap()])
            if SA >= 3:
                stage(e_full[0].ap(), gB, colB, wB, sB, a_dense[0], NT_B)
            if SA >= 4:
                w_phase(a_dense[0], W1, b1rep, h1, None)

            # ---------------- layer 2
            if SA >= 5:
                dram_memset(e_part[1], E_PAD)
                dram_memset(a_dense[1], NPC_PAD)
                stage(h1.ap(), gA, colA, wA, sA, e_part[1], NT_A)
                nc.gpsimd.collective_compute(
                    "AllReduce", mybir.AluOpType.add,
                    replica_groups=[list(range(NCORES))],
                    ins=[e_part[1].ap()], outs=[e_full[1].ap()])
                stage(e_full[1].ap(), gB, colB, wB, sB, a_dense[1], NT_B)
            pool_acc = pspool.tile([128, N_GRAPHS], F32, tag="pool")
            if SA >= 6:
                w_phase(a_dense[1], W2, b2rep, None, pool_acc[:])
            else:
                nc.vector.tensor_copy(pool_acc[:], cntinvrep[:])

            # ---------------- pooling + fc + log_softmax
            pt = wp.tile([128, N_GRAPHS], F32, tag="pt")
            nc.vector.tensor_tensor(pt[:], pool_acc[:], cntinvrep[:],
                                    op=mybir.AluOpType.mult)
            nc.sync.dma_start(pool_part.ap(), pt[:])
            nc.gpsimd.collective_compute(
                "AllReduce", mybir.AluOpType.add,
                replica_groups=[list(range(NCORES))],
                ins=[pool_part.ap()], outs=[pool_full.ap()])
            ptf = wp.tile([128, N_GRAPHS], F32, tag="ptf")
            nc.sync.dma_start(ptf[:], pool_full.ap())
            lg = pspool.tile([N_GRAPHS, OUT_C], F32, tag="lg")
            nc.tensor.matmul(lg[:], ptf[:], Wfc[:], start=True, stop=True)
            z = wp.tile([N_GRAPHS, OUT_C], F32, tag="z")
            nc.vector.tensor_tensor(z[:], lg[:], bfcrep[:],
                                    op=mybir.AluOpType.add)
            mx = wp.tile([N_GRAPHS, 1], F32, tag="mx")
            nc.vector.tensor_reduce(mx[:], z[:], mybir.AxisListType.X,
                                    mybir.AluOpType.max)
            zs = wp.tile([N_GRAPHS, OUT_C], F32, tag="zs")
            nc.vector.tensor_scalar(zs[:], z[:], mx[:], None,
                                    op0=mybir.AluOpType.subtract)
            ez = wp.tile([N_GRAPHS, OUT_C], F32, tag="ez")
            se = wp.tile([N_GRAPHS, 1], F32, tag="se")
            nc.scalar.activation(ez[:], zs[:], mybir.ActivationFunctionType.Exp,
                                 accum_out=se[:])
            lse = wp.tile([N_GRAPHS, 1], F32, tag="lse")
            nc.scalar.activation(lse[:], se[:], mybir.ActivationFunctionType.Ln)
            outz = wp.tile([N_GRAPHS, OUT_C], F32, tag="outz")
            nc.vector.tensor_scalar(outz[:], zs[:], lse[:], None,
                                    op0=mybir.AluOpType.subtract)
            nc.sync.dma_start(out_t.ap(), outz[:])

    nc.compile()
    return nc


_CACHE = {}


def kernel(**inputs) -> np.ndarray:
    in_maps, meta = _prep_inputs(**inputs)
    key = (meta["NT_A"], meta["NT_B"])
    if key not in _CACHE:
        _CACHE[key] = _build(meta)
    nc = _CACHE[key]
    res = run_bass_kernel_spmd(nc, in_maps, core_ids=list(range(NCORES)))
    return res.results[0]["out"].astype(np.float32)


if __name__ == "__main__":
    rng = np.random.default_rng(0)
    ins = dict(
        x=rng.standard_normal((N_NODES, C)).astype(np.float32),
        node_idx=rng.integers(0, N_NODES, NNZ),
        edge_idx=rng.integers(0, N_EDGES, NNZ),
        batch=np.sort(rng.integers(0, N_GRAPHS, N_NODES)),
        W1=rng.uniform(-0.09, 0.09, (C, C)).astype(np.float32),
        b1=np.zeros(C, np.float32),
        W2=rng.uniform(-0.09, 0.09, (C, C)).astype(np.float32),
        b2=np.zeros(C, np.float32),
        Wfc=rng.uniform(-0.09, 0.09, (C, OUT_C)).astype(np.float32),
        bfc=rng.uniform(-0.09, 0.09, OUT_C).astype(np.float32),
    )
    out = kernel(**ins)
    print("out", out.shape, out[:2, :4])

